# revision 5
# baseline (speedup 1.0000x reference)
"""Trainium2 Bass kernel for a 4-layer dense transformer with tied-embedding head.

Distribution across 8 NeuronCores:
  - Sequence-parallel transformer: core c owns tokens [256c, 256c+256).
    Weights replicated; K/V (plus a ones-column for softmax sums) are
    AllGathered per layer.  Causality is enforced with multiplicative
    per-(core, kv-block) masks shipped as data, so the SPMD program is
    identical on every core.
  - Vocab-parallel logits: after the final LN, the (transposed) hidden
    states are AllGathered and each core computes logits against its
    4000-row slice of the tied embedding; the host concatenates.

All matmuls run in bf16 on the TensorEngine (fp32 PSUM accumulation);
LayerNorm statistics, softmax sums and the residual stream are fp32.
"""

import os
import numpy as np
import ml_dtypes

BF = ml_dtypes.bfloat16

N_CORES = 8
L, D, H, HD, DF, V, T = 4, 1024, 16, 64, 4096, 32000, 2048
TC = T // N_CORES            # 256 tokens per core
VC = V // N_CORES            # 4000 vocab rows per core
P = 128
NB = T // P                  # 16 kv blocks of 128 tokens
EPS = 1e-5
SCALE = 1.0 / 8.0            # 1/sqrt(HD)

_CACHE = {}

LAST_EXEC_NS = None
LAST_RESULTS = None


def _build(n_layers=L, debug_taps=False):
    import concourse.bass as bass
    import concourse.mybir as mybir
    import concourse.tile as tile
    from concourse import bacc
    from concourse.masks import make_identity

    dt = mybir.dt
    AF = mybir.ActivationFunctionType
    OP = mybir.AluOpType
    AX = mybir.AxisListType

    nc = bacc.Bacc("TRN2", target_bir_lowering=False, debug=False,
                   num_devices=N_CORES)

    # ---- external inputs (per-core) ----
    te_b = nc.declare_dram_parameter("te_b", [V, D], dt.bfloat16, isOutput=False)
    teT_b = nc.declare_dram_parameter("teT_b", [D, VC], dt.bfloat16, isOutput=False)
    pe_c = nc.declare_dram_parameter("pe_c", [TC, D], dt.bfloat16, isOutput=False)
    idx_c = nc.declare_dram_parameter("idx_c", [TC, 1], dt.int32, isOutput=False)
    w_qkv = nc.declare_dram_parameter("w_qkv", [L, D, 3 * D], dt.bfloat16, isOutput=False)
    w_out = nc.declare_dram_parameter("w_out", [L, D, D], dt.bfloat16, isOutput=False)
    w_up = nc.declare_dram_parameter("w_up", [L, D, DF], dt.bfloat16, isOutput=False)
    w_dn = nc.declare_dram_parameter("w_dn", [L, DF, D], dt.bfloat16, isOutput=False)
    b_qkv = nc.declare_dram_parameter("b_qkv", [L, 3 * D], dt.float32, isOutput=False)
    b_up = nc.declare_dram_parameter("b_up", [L, DF], dt.float32, isOutput=False)
    masks = nc.declare_dram_parameter("masks", [NB, P, TC], dt.bfloat16, isOutput=False)

    logits_out = nc.declare_dram_parameter("logits", [T, VC], dt.float32, isOutput=True)
    dbg = None
    if debug_taps:
        dbg = nc.declare_dram_parameter("dbg", [n_layers + 1, TC, D], dt.float32,
                                        isOutput=True)

    KT_ELEMS = D * TC                   # kT chunk, [D, TC] feature-major
    VE_ELEMS = TC * H * (HD + 1)        # v'' chunk, [TC, H*65] token-major
    CHUNK = KT_ELEMS + VE_ELEMS

    rg = [list(range(N_CORES))]

    with tile.TileContext(nc) as tc:
        with (
            tc.tile_pool(name="sb", bufs=1) as sb,
            tc.tile_pool(name="ps", bufs=7, space="PSUM") as psp,
            tc.tile_pool(name="dram", bufs=1, space="DRAM") as dram,
        ):
            def ptile(name):
                return psp.tile([P, 512], dt.float32, tag="ps", bufs=7, name=name)

            def ptile_bf(name):
                # bf16 psum view for PE-transpose outputs (pass-through dtype)
                return psp.tile([P, 512], dt.bfloat16, tag="ps", bufs=7, name=name)

            ident = sb.tile([P, P], dt.bfloat16, tag="ident", name="ident")
            make_identity(nc, ident[:])
            eps_sb = sb.tile([P, 1], dt.float32, tag="eps", name="eps_sb")
            nc.vector.memset(eps_sb[:], EPS)

            mask_sb = sb.tile([P, NB, TC], dt.bfloat16, tag="mask", name="mask_sb")
            for g in range(NB):
                nc.sync.dma_start(mask_sb[:, g, :], masks[g])

            # ---- embedding: resid[mt] = te[ids] + pe ----
            resid = []
            for mt in range(2):
                idx_sb = sb.tile([P, 1], dt.int32, tag="idx", bufs=2, name=f"idx{mt}")
                nc.sync.dma_start(idx_sb[:], idx_c[mt * P:(mt + 1) * P, :])
                emb = sb.tile([P, D], dt.bfloat16, tag="scratch4", bufs=2, name=f"emb{mt}")
                nc.gpsimd.indirect_dma_start(
                    out=emb[:], out_offset=None, in_=te_b[:],
                    in_offset=bass.IndirectOffsetOnAxis(ap=idx_sb[:, :1], axis=0),
                )
                pe_sb = sb.tile([P, D], dt.bfloat16, tag="scratch4", bufs=2, name=f"pe{mt}")
                nc.sync.dma_start(pe_sb[:], pe_c[mt * P:(mt + 1) * P, :])
                r = sb.tile([P, D], dt.float32, tag=f"resid{mt}", name=f"resid{mt}")
                nc.vector.tensor_tensor(out=r[:], in0=emb[:], in1=pe_sb[:], op=OP.add)
                resid.append(r)

            if debug_taps:
                for mt in range(2):
                    nc.sync.dma_start(dbg[0, mt * P:(mt + 1) * P, :], resid[mt][:])

            def layernorm_into(x_f32, out_bf, name):
                """out_bf = (x - mean(x)) * rsqrt(var(x) + eps), fp32 stats."""
                s = sb.tile([P, 1], dt.float32, tag="lnstat", bufs=8, name=f"{name}_s")
                nc.vector.reduce_sum(out=s[:], in_=x_f32[:], axis=AX.X)
                sq = sb.tile([P, D], dt.float32, tag="scratch4", bufs=2, name=f"{name}_sq")
                ssq = sb.tile([P, 1], dt.float32, tag="lnstat", bufs=8, name=f"{name}_ssq")
                nc.scalar.activation(sq[:], x_f32[:], AF.Square, accum_out=ssq[:])
                nmean = sb.tile([P, 1], dt.float32, tag="lnstat", bufs=8, name=f"{name}_nm")
                nc.vector.tensor_scalar(out=nmean[:], in0=s[:], scalar1=-1.0 / D,
                                        scalar2=None, op0=OP.mult)
                # var = ssq/D - mean^2 ; std = sqrt(var + eps)
                msq = sb.tile([P, 1], dt.float32, tag="lnstat", bufs=8, name=f"{name}_msq")
                nc.vector.tensor_tensor(out=msq[:], in0=nmean[:], in1=nmean[:], op=OP.mult)
                var = sb.tile([P, 1], dt.float32, tag="lnstat", bufs=8, name=f"{name}_var")
                nc.vector.tensor_scalar(out=var[:], in0=ssq[:], scalar1=1.0 / D,
                                        scalar2=None, op0=OP.mult)
                nc.vector.tensor_tensor(out=var[:], in0=var[:], in1=msq[:],
                                        op=OP.subtract)
                std = sb.tile([P, 1], dt.float32, tag="lnstat", bufs=8, name=f"{name}_std")
                nc.scalar.activation(std[:], var[:], AF.Sqrt, bias=eps_sb[:])
                rstd = sb.tile([P, 1], dt.float32, tag="lnstat", bufs=8, name=f"{name}_rstd")
                nc.vector.reciprocal(rstd[:], std[:])
                nc.vector.tensor_scalar(out=out_bf[:], in0=x_f32[:], scalar1=nmean[:],
                                        scalar2=rstd[:], op0=OP.add, op1=OP.mult)

            def ln_and_transpose(li, which):
                """LayerNorm resid -> xlnT [128, 8, 256] bf16 (feature-major)."""
                xlnT = sb.tile([P, D // P, TC], dt.bfloat16, tag="xlnT", bufs=2,
                               name=f"xlnT_{li}_{which}")
                for mt in range(2):
                    xln = sb.tile([P, D], dt.bfloat16, tag="xln", bufs=2,
                                  name=f"xln_{li}_{which}_{mt}")
                    layernorm_into(resid[mt], xln, f"ln{which}_{li}_{mt}")
                    for kb in range(D // P):
                        pt = ptile_bf(f"tp_{li}_{which}_{mt}_{kb}")
                        nc.tensor.transpose(pt[:, 0:P], xln[:, kb * P:(kb + 1) * P],
                                            ident[:])
                        nc.scalar.activation(xlnT[:, kb, mt * P:(mt + 1) * P],
                                             pt[:, 0:P], AF.Identity, bias=0.0)
                return xlnT

            ccouts = []

            for li in range(n_layers):
                # ---------------- attention ----------------
                xlnT = ln_and_transpose(li, 1)

                # Wqk resident [128, 8kb, 2048]
                wqk = sb.tile([P, D // P, 2 * D], dt.bfloat16, tag="big32",
                              name=f"wqk_{li}")
                for kb in range(D // P):
                    nc.sync.dma_start(wqk[:, kb, :],
                                      w_qkv[li, kb * P:(kb + 1) * P, 0:2 * D])
                qkvb = sb.tile([P, 24], dt.float32, tag="qkvb", bufs=2,
                               name=f"qkvb_{li}")
                nc.sync.dma_start(
                    qkvb[:], b_qkv[li].rearrange("(a p) -> p a", p=P))

                qT = sb.tile([P, D // P, TC], dt.bfloat16, tag="qT", bufs=1,
                             name=f"qT_{li}")
                kT = sb.tile([P, D // P, TC], dt.bfloat16, tag="kT", bufs=1,
                             name=f"kT_{li}")
                for ft in range(16):
                    pq = ptile(f"pqk_{li}_{ft}")
                    for kb in range(D // P):
                        nc.tensor.matmul(pq[:, 0:TC],
                                         lhsT=wqk[:, kb, ft * P:(ft + 1) * P],
                                         rhs=xlnT[:, kb, :],
                                         start=(kb == 0), stop=(kb == D // P - 1))
                    dstT = qT if ft < 8 else kT
                    slot = ft if ft < 8 else ft - 8
                    nc.scalar.activation(dstT[:, slot, :], pq[:, 0:TC], AF.Identity,
                                         bias=qkvb[:, ft:ft + 1])

                # v (token-major) with ones column -> v'' [128, 16h, 65]
                vloc = [sb.tile([P, H, HD + 1], dt.bfloat16, tag="vloc", bufs=2,
                                name=f"vloc_{li}_{mt}") for mt in range(2)]
                pv = [[None] * 2 for _ in range(2)]
                for mt in range(2):
                    for nt in range(2):
                        pv[mt][nt] = ptile(f"pv_{li}_{mt}_{nt}")
                for kb in range(D // P):
                    wv = sb.tile([P, D], dt.bfloat16, tag="wstream", bufs=4,
                                 name=f"wv_{li}_{kb}")
                    nc.sync.dma_start(wv[:], w_qkv[li, kb * P:(kb + 1) * P,
                                                   2 * D:3 * D])
                    for mt in range(2):
                        for nt in range(2):
                            nc.tensor.matmul(pv[mt][nt][:, 0:512],
                                             lhsT=xlnT[:, kb, mt * P:(mt + 1) * P],
                                             rhs=wv[:, nt * 512:(nt + 1) * 512],
                                             start=(kb == 0), stop=(kb == D // P - 1))
                for mt in range(2):
                    for nt in range(2):
                        nc.scalar.activation(
                            vloc[mt][:, nt * 8:(nt + 1) * 8, 0:HD],
                            pv[mt][nt][:, 0:512].rearrange("p (h d) -> p h d", h=8),
                            AF.Identity, bias=0.0)
                    nc.vector.memset(vloc[mt][:, :, HD:HD + 1], 1.0)

                # ---- AllGather kT + v'' ----
                cc_in = dram.tile([CHUNK], dt.bfloat16, tag="ccin", bufs=2,
                                  name=f"ccin_{li}")
                cc_out = dram.tile([N_CORES * CHUNK], dt.bfloat16, tag="ccout",
                                   bufs=2, addr_space="Shared", name=f"ccout_{li}")
                kt_dst = cc_in[0:KT_ELEMS].rearrange("(f t) -> f t", t=TC)
                for kb in range(D // P):
                    nc.scalar.dma_start(kt_dst[kb * P:(kb + 1) * P, :], kT[:, kb, :])
                ve_dst = cc_in[KT_ELEMS:CHUNK].rearrange("(t f) -> t f",
                                                         f=H * (HD + 1))
                for mt in range(2):
                    nc.scalar.dma_start(
                        ve_dst[mt * P:(mt + 1) * P, :],
                        vloc[mt][:].rearrange("p h d -> p (h d)"))
                nc.gpsimd.collective_compute(
                    "AllGather", mybir.AluOpType.bypass, replica_groups=rg,
                    ins=[cc_in[:]], outs=[cc_out[:]],
                )
                ccouts.append(cc_out)

                # up_w resident (prefetch during attention)
                upw = sb.tile([P, D // P, DF], dt.bfloat16, tag="big64",
                              name=f"upw_{li}")
                for kb in range(D // P):
                    nc.sync.dma_start(upw[:, kb, :],
                                      w_up[li, kb * P:(kb + 1) * P, :])
                upb = sb.tile([P, DF // P], dt.float32, tag="upb", bufs=2,
                              name=f"upb_{li}")
                nc.sync.dma_start(upb[:], b_up[li].rearrange("(a p) -> p a", p=P))

                # ---- attention proper ----
                aoT = sb.tile([P, D // P, TC], dt.bfloat16, tag="aoT", bufs=2,
                              name=f"aoT_{li}")
                for hp in range(H // 2):
                    pao = [[None] * 2 for _ in range(2)]  # [j][mt] -> [128q, 65]
                    for j in range(2):
                        for mt in range(2):
                            pao[j][mt] = ptile(f"pao_{li}_{hp}_{j}_{mt}")
                    for g in range(NB):
                        c2, half = g // 2, g % 2
                        kt2 = sb.tile([P, P], dt.bfloat16, tag="kt2", bufs=4,
                                      name=f"kt2_{li}_{hp}_{g}")
                        ksrc = ccouts[li][c2 * CHUNK:c2 * CHUNK + KT_ELEMS] \
                            .rearrange("(f t) -> f t", t=TC)
                        nc.sync.dma_start(
                            kt2[:], ksrc[hp * P:(hp + 1) * P,
                                         half * P:(half + 1) * P])
                        vsrc = ccouts[li][c2 * CHUNK + KT_ELEMS:(c2 + 1) * CHUNK] \
                            .rearrange("(t f) -> t f", f=H * (HD + 1))
                        for j in range(2):
                            h = 2 * hp + j
                            lo, hi = j * HD, (j + 1) * HD
                            psc = ptile(f"psc_{li}_{hp}_{g}_{j}")
                            nc.tensor.matmul(psc[:, 0:TC], lhsT=kt2[lo:hi, :],
                                             rhs=qT[lo:hi, hp, :],
                                             start=True, stop=True)
                            pexp = sb.tile([P, TC], dt.bfloat16, tag="pexp", bufs=6,
                                           name=f"pexp_{li}_{hp}_{g}_{j}")
                            nc.scalar.activation(pexp[:], psc[:, 0:TC], AF.Exp,
                                                 scale=SCALE)
                            nc.vector.tensor_tensor(out=pexp[:], in0=pexp[:],
                                                    in1=mask_sb[:, g, :], op=OP.mult)
                            vh = sb.tile([P, HD + 1], dt.bfloat16, tag="vh", bufs=6,
                                         name=f"vh_{li}_{hp}_{g}_{j}")
                            nc.sync.dma_start(
                                vh[:], vsrc[half * P:(half + 1) * P,
                                            h * (HD + 1):(h + 1) * (HD + 1)])
                            for mt in range(2):
                                nc.tensor.matmul(
                                    pao[j][mt][:, 0:HD + 1],
                                    lhsT=pexp[:, mt * P:(mt + 1) * P],
                                    rhs=vh[:],
                                    start=(g == 0), stop=(g == NB - 1))
                    # normalize + transpose into aoT
                    for mt in range(2):
                        aon = sb.tile([P, P], dt.bfloat16, tag="aon", bufs=4,
                                      name=f"aon_{li}_{hp}_{mt}")
                        for j in range(2):
                            rs = sb.tile([P, 1], dt.float32, tag="recip", bufs=8,
                                         name=f"rs_{li}_{hp}_{mt}_{j}")
                            nc.vector.reciprocal(rs[:], pao[j][mt][:, HD:HD + 1])
                            nc.vector.tensor_scalar(
                                out=aon[:, j * HD:(j + 1) * HD],
                                in0=pao[j][mt][:, 0:HD],
                                scalar1=rs[:], scalar2=None, op0=OP.mult)
                        pt = ptile_bf(f"paot_{li}_{hp}_{mt}")
                        nc.tensor.transpose(pt[:, 0:P], aon[:], ident[:])
                        nc.scalar.activation(aoT[:, hp, mt * P:(mt + 1) * P],
                                             pt[:, 0:P], AF.Identity, bias=0.0)

                # ---- out projection + residual ----
                po = [[None] * 2 for _ in range(2)]
                for mt in range(2):
                    for nt in range(2):
                        po[mt][nt] = ptile(f"po_{li}_{mt}_{nt}")
                for kb in range(D // P):
                    wob = sb.tile([P, D], dt.bfloat16, tag="wstream", bufs=4,
                                  name=f"wo_{li}_{kb}")
                    nc.sync.dma_start(wob[:], w_out[li, kb * P:(kb + 1) * P, :])
                    for mt in range(2):
                        for nt in range(2):
                            nc.tensor.matmul(po[mt][nt][:, 0:512],
                                             lhsT=aoT[:, kb, mt * P:(mt + 1) * P],
                                             rhs=wob[:, nt * 512:(nt + 1) * 512],
                                             start=(kb == 0), stop=(kb == D // P - 1))
                for mt in range(2):
                    for nt in range(2):
                        nc.vector.tensor_tensor(
                            out=resid[mt][:, nt * 512:(nt + 1) * 512],
                            in0=resid[mt][:, nt * 512:(nt + 1) * 512],
                            in1=po[mt][nt][:, 0:512], op=OP.add)

                # ---------------- MLP ----------------
                xlnT2 = ln_and_transpose(li, 2)
                hT = sb.tile([P, DF // P, TC], dt.bfloat16, tag="hT",
                             name=f"hT_{li}")
                for ft in range(DF // P):
                    ph = ptile(f"ph_{li}_{ft}")
                    for kb in range(D // P):
                        nc.tensor.matmul(ph[:, 0:TC],
                                         lhsT=upw[:, kb, ft * P:(ft + 1) * P],
                                         rhs=xlnT2[:, kb, :],
                                         start=(kb == 0), stop=(kb == D // P - 1))
                    nc.scalar.activation(hT[:, ft, :], ph[:, 0:TC], AF.Silu,
                                         bias=upb[:, ft:ft + 1])

                pd = [[None] * 2 for _ in range(2)]
                for mt in range(2):
                    for nt in range(2):
                        pd[mt][nt] = ptile(f"pd_{li}_{mt}_{nt}")
                for kb in range(DF // P):
                    wd = sb.tile([P, D], dt.bfloat16, tag="wstream", bufs=4,
                                 name=f"wd_{li}_{kb}")
                    nc.sync.dma_start(wd[:], w_dn[li, kb * P:(kb + 1) * P, :])
                    for mt in range(2):
                        for nt in range(2):
                            nc.tensor.matmul(pd[mt][nt][:, 0:512],
                                             lhsT=hT[:, kb, mt * P:(mt + 1) * P],
                                             rhs=wd[:, nt * 512:(nt + 1) * 512],
                                             start=(kb == 0), stop=(kb == DF // P - 1))
                for mt in range(2):
                    for nt in range(2):
                        nc.vector.tensor_tensor(
                            out=resid[mt][:, nt * 512:(nt + 1) * 512],
                            in0=resid[mt][:, nt * 512:(nt + 1) * 512],
                            in1=pd[mt][nt][:, 0:512], op=OP.add)

                if debug_taps:
                    for mt in range(2):
                        nc.sync.dma_start(dbg[li + 1, mt * P:(mt + 1) * P, :],
                                          resid[mt][:])

            # ---------------- final LN + logits ----------------
            xfT = ln_and_transpose(n_layers, 0)  # reuses xlnT tag
            cc2_in = dram.tile([KT_ELEMS], dt.bfloat16, tag="cc2in",
                               name="cc2_in")
            cc2_out = dram.tile([N_CORES * KT_ELEMS], dt.bfloat16, tag="cc2out",
                                addr_space="Shared", name="cc2_out")
            x_dst = cc2_in[:].rearrange("(f t) -> f t", t=TC)
            for kb in range(D // P):
                nc.scalar.dma_start(x_dst[kb * P:(kb + 1) * P, :], xfT[:, kb, :])
            nc.gpsimd.collective_compute(
                "AllGather", mybir.AluOpType.bypass, replica_groups=rg,
                ins=[cc2_in[:]], outs=[cc2_out[:]],
            )

            xall = sb.tile([P, N_CORES * (D // P), TC], dt.bfloat16, tag="big32",
                           name="xall")
            for c2 in range(N_CORES):
                xsrc = cc2_out[c2 * KT_ELEMS:(c2 + 1) * KT_ELEMS] \
                    .rearrange("(f t) -> f t", t=TC)
                for kb in range(D // P):
                    nc.sync.dma_start(xall[:, c2 * (D // P) + kb, :],
                                      xsrc[kb * P:(kb + 1) * P, :])

            teT_sb = sb.tile([P, D // P, VC], dt.bfloat16, tag="big64",
                             name="teT_sb")
            for kb in range(D // P):
                nc.sync.dma_start(teT_sb[:, kb, :],
                                  teT_b[kb * P:(kb + 1) * P, :])

            NTS = [512] * 7 + [VC - 7 * 512]  # 4000 = 7*512 + 416
            for c2 in range(N_CORES):
                for mt in range(2):
                    for nt in range(8):
                        n0 = nt * 512
                        nn = NTS[nt]
                        pl = ptile(f"pl_{c2}_{mt}_{nt}")
                        for kb in range(D // P):
                            nc.tensor.matmul(
                                pl[:, 0:nn],
                                lhsT=xall[:, c2 * (D // P) + kb,
                                          mt * P:(mt + 1) * P],
                                rhs=teT_sb[:, kb, n0:n0 + nn],
                                start=(kb == 0), stop=(kb == D // P - 1))
                        lo = sb.tile([P, 512], dt.float32, tag="lout", bufs=2,
                                     name=f"lo_{c2}_{mt}_{nt}")
                        if nt % 2 == 0:
                            nc.scalar.activation(lo[:, 0:nn], pl[:, 0:nn],
                                                 AF.Identity, bias=0.0)
                        else:
                            nc.vector.tensor_copy(lo[:, 0:nn], pl[:, 0:nn])
                        nc.sync.dma_start(
                            logits_out[c2 * TC + mt * P:c2 * TC + (mt + 1) * P,
                                       n0:n0 + nn],
                            lo[:, 0:nn])

    nc.compile()
    return nc


def _get_program(n_layers, debug_taps):
    key = (n_layers, debug_taps)
    if key not in _CACHE:
        _CACHE[key] = _build(n_layers, debug_taps)
    return _CACHE[key]


def _host_prep(inputs, n_layers):
    ids = np.asarray(inputs["ids"]).reshape(-1).astype(np.int32)   # [T]
    te = np.asarray(inputs["te"], dtype=np.float32)
    pe = np.asarray(inputs["pe"], dtype=np.float32)

    for nm in ("qkv_b", "out_b", "up_b", "dn_b"):
        pass  # qkv_b/up_b fused on device; out_b/dn_b handled below

    # trivial-parameter checks (spec fills: biases zero, ln weights one)
    assert not np.any(np.asarray(inputs["qkv_b"])[:, 2 * D:]), "nonzero v bias unsupported"
    assert not np.any(np.asarray(inputs["out_b"])), "nonzero out_b unsupported"
    assert not np.any(np.asarray(inputs["dn_b"])), "nonzero dn_b unsupported"
    for nm in ("ln1_w", "ln2_w"):
        assert np.all(np.asarray(inputs[nm]) == 1.0), f"non-unit {nm} unsupported"
    for nm in ("ln1_b", "ln2_b"):
        assert not np.any(np.asarray(inputs[nm])), f"nonzero {nm} unsupported"
    assert np.all(np.asarray(inputs["lnf_w"]) == 1.0), "non-unit lnf_w unsupported"
    assert not np.any(np.asarray(inputs["lnf_b"])), "nonzero lnf_b unsupported"

    te_b = te.astype(BF)
    w_qkv = np.asarray(inputs["qkv_w"], np.float32).astype(BF)
    w_out = np.asarray(inputs["out_w"], np.float32).astype(BF)
    w_up = np.asarray(inputs["up_w"], np.float32).astype(BF)
    w_dn = np.asarray(inputs["dn_w"], np.float32).astype(BF)
    b_qkv = np.asarray(inputs["qkv_b"], np.float32)
    b_up = np.asarray(inputs["up_b"], np.float32)

    in_maps = []
    for c in range(N_CORES):
        t0 = c * TC
        # causal masks per kv block g: [128 kv, 256 q] wrt my tokens
        m = np.zeros((NB, P, TC), np.float32)
        r = np.arange(P)[:, None]
        q = np.arange(TC)[None, :]
        for g in range(NB):
            m[g] = ((g * P + r) <= (t0 + q))
        in_maps.append({
            "te_b": te_b,
            "teT_b": np.ascontiguousarray(te[c * VC:(c + 1) * VC].T).astype(BF),
            "pe_c": pe[t0:t0 + TC].astype(BF),
            "idx_c": ids[t0:t0 + TC].reshape(TC, 1),
            "w_qkv": w_qkv, "w_out": w_out, "w_up": w_up, "w_dn": w_dn,
            "b_qkv": b_qkv, "b_up": b_up,
            "masks": m.astype(BF),
        })
    return in_maps


def kernel(**inputs):
    global LAST_EXEC_NS, LAST_RESULTS
    from concourse.bass_utils import run_bass_kernel_spmd

    n_layers = int(os.environ.get("KERNEL_LAYERS", L))
    debug_taps = os.environ.get("KERNEL_DEBUG_TAPS", "") == "1"
    profile = os.environ.get("KERNEL_PROFILE", "") == "1"
    if profile:
        _install_profhook()

    nc = _get_program(n_layers, debug_taps)
    in_maps = _host_prep(inputs, n_layers)
    res = run_bass_kernel_spmd(nc, in_maps, core_ids=list(range(N_CORES)),
                               trace=profile)
    LAST_EXEC_NS = res.exec_time_ns
    LAST_RESULTS = res
    logits = np.concatenate([res.results[c]["logits"] for c in range(N_CORES)],
                            axis=1)
    return logits.reshape(1, T, V).astype(np.float32)


def _install_profhook():
    import contextlib
    import ctypes
    import sys
    import types

    if "antenv.axon_hooks" in sys.modules:
        return
    try:
        lib = ctypes.CDLL("/opt/axon/libaxon_pjrt.so")
        if not hasattr(lib, "axon_start_nrt_profile"):
            return
    except OSError:
        return
    lib.axon_start_nrt_profile.argtypes = [ctypes.POINTER(ctypes.c_int64),
                                           ctypes.c_size_t]
    lib.axon_start_nrt_profile.restype = ctypes.c_int64
    lib.axon_stop_nrt_profile.argtypes = [ctypes.c_char_p]
    lib.axon_stop_nrt_profile.restype = ctypes.c_int64

    @contextlib.contextmanager
    def _hook(output_dir, device_ids):
        import jax
        jax.devices()
        if device_ids:
            ids = (ctypes.c_int64 * len(device_ids))(*device_ids)
            rc = lib.axon_start_nrt_profile(ids, len(device_ids))
        else:
            rc = lib.axon_start_nrt_profile(None, 0)
        if rc != 0:
            raise RuntimeError(f"axon_start_nrt_profile rc={rc}")
        try:
            yield
        finally:
            n = lib.axon_stop_nrt_profile(str(output_dir).encode())
            print(f"profile: {n} file(s) written to {output_dir}",
                  file=sys.stderr)

    mod = types.ModuleType("antenv.axon_hooks")
    mod.get_axon_ntff_profile_hook = lambda: _hook
    mod.set_axon_ntff_profile_hook = lambda h: None
    sys.modules["antenv.axon_hooks"] = mod
